# revision 4
# baseline (speedup 1.0000x reference)
"""Bidirectional attention kernel for Trainium2 (Bass/Tile), SPMD over 8 cores.

Per batch n (one batch per core):
    scores  = (lhs * w_lhs) @ (rhs * w_rhs).T          # [L, R]
            = (lhs * (w_lhs*w_rhs)) @ rhs.T            # diagonal scales compose
    E       = exp(scores)                              # no max-subtraction:
                                                       # |scores| < ~0.5 for these inputs
    lhs_ctx = (E @ rhs)   / rowsum(E)                  # row softmax folded into scale
    rhs_ctx = (E.T @ lhs) / colsum(E)                  # col softmax folded into scale
    out_lhs = [lhs | lhs_ctx],  out_rhs = [rhs | rhs_ctx]

Matmuls run in bf16 (fp32 accumulate in PSUM). All transposes use the DMA
XBAR (bf16 SBUF->SBUF), keeping the tensor engine free for matmuls.
E stays resident in SBUF ([128, L/128, R] bf16)."""

import os
import sys

import numpy as np

for _p in ("/root/.axon_site/_ro/trn_rl_repo", "/opt/trn_rl_repo"):
    if os.path.isdir(_p) and _p not in sys.path:
        sys.path.append(_p)

N_CORES = 8
L, R, D = 2048, 2048, 1024


def build_program(L, R, D):
    from contextlib import ExitStack

    import concourse.bass as bass
    import concourse.mybir as mybir
    import concourse.tile as tile
    from concourse import bacc

    f32 = mybir.dt.float32
    bf16 = mybir.dt.bfloat16
    P = 128
    LC, RC, DC = L // P, R // P, D // P
    SW = min(1024, R)      # S-psum tile width (fp32, 2 banks)
    NS = R // SW           # number of S-psum tiles per l-chunk
    MN = min(512, D)       # matmul moving free dim (one PSUM bank of fp32)
    MNS = min(512, SW)     # same, for the scores matmul

    nc = bacc.Bacc("TRN2", target_bir_lowering=False, debug=False)

    lhs = nc.dram_tensor("lhs", [L, D], f32, kind="ExternalInput")
    rhs = nc.dram_tensor("rhs", [R, D], f32, kind="ExternalInput")
    w_lhs = nc.dram_tensor("w_lhs", [1, D], f32, kind="ExternalInput")
    w_rhs = nc.dram_tensor("w_rhs", [1, D], f32, kind="ExternalInput")
    out_lhs = nc.dram_tensor("out_lhs", [L, 2 * D], f32, kind="ExternalOutput")
    out_rhs = nc.dram_tensor("out_rhs", [R, 2 * D], f32, kind="ExternalOutput")

    Exp = mybir.ActivationFunctionType.Exp
    Copy = mybir.ActivationFunctionType.Copy
    mult = mybir.AluOpType.mult
    add = mybir.AluOpType.add

    with tile.TileContext(nc) as tc, ExitStack() as ctx:
        const = ctx.enter_context(tc.tile_pool(name="const", bufs=1))
        res = ctx.enter_context(tc.tile_pool(name="res", bufs=1))
        dram = ctx.enter_context(tc.tile_pool(name="dram", bufs=1, space="DRAM"))
        inp = ctx.enter_context(tc.tile_pool(name="inp", bufs=2))
        work = ctx.enter_context(tc.tile_pool(name="work", bufs=2))
        outp = ctx.enter_context(tc.tile_pool(name="outp", bufs=1))
        scal = ctx.enter_context(tc.tile_pool(name="scal", bufs=3))

        # Resident tensors
        E = res.tile([P, LC, R], bf16)       # exp(scores), natural [l, r]
        T2 = res.tile([P, DC, R], bf16)      # rhs^T  [d, r]
        rhsb = res.tile([P, RC, D], bf16)    # rhs natural, bf16
        lhsb = res.tile([P, LC, D], bf16)    # lhs natural, bf16
        w2T = const.tile([P, DC], f32)       # (w_lhs*w_rhs) in [d%128, d//128]
        onesb = const.tile([P, 1], bf16)
        nc.vector.memset(onesb[:], 1.0)

        # w2 = w_lhs * w_rhs, staged through DRAM to land in [di, dc] layout
        wl_s = const.tile([1, D], f32)
        wr_s = const.tile([1, D], f32)
        w2_s = const.tile([1, D], f32)
        w2_d = dram.tile([1, D], f32)
        nc.sync.dma_start(wl_s[:], w_lhs[:])
        nc.sync.dma_start(wr_s[:], w_rhs[:])
        nc.vector.tensor_mul(w2_s[:], wl_s[:], wr_s[:])
        nc.sync.dma_start(w2_d[:], w2_s[:])
        nc.sync.dma_start(w2T[:], w2_d[0, :].rearrange("(dc di) -> di dc", di=P))

        # Phase A: rhs-side prep
        for k in range(RC):
            rin = inp.tile([P, D], f32, tag="inf32")
            nc.sync.dma_start(rin[:], rhs[k * P:(k + 1) * P, :])
            nc.sync.dma_start(out_rhs[k * P:(k + 1) * P, 0:D], rin[:])
            nc.vector.tensor_copy(rhsb[:, k, :], rin[:])
            nc.sync.dma_start_transpose(T2[:, :, k * P:(k + 1) * P], rhsb[:, k, :])

        # Phase B: per l-chunk: scores -> E -> E^T -> lhs_ctx
        with tc.tile_pool(name="psS", bufs=2, space="PSUM") as psS, \
             tc.tile_pool(name="psC1", bufs=2, space="PSUM") as psC1:
            for i in range(LC):
                lin = inp.tile([P, D], f32, tag="inf32")
                nc.sync.dma_start(lin[:], lhs[i * P:(i + 1) * P, :])
                nc.sync.dma_start(out_lhs[i * P:(i + 1) * P, 0:D], lin[:])
                nc.vector.tensor_copy(lhsb[:, i, :], lin[:])

                lsb = work.tile([P, D], bf16, tag="lsb")
                nc.vector.tensor_copy(lsb[:], lin[:])
                T1 = work.tile([P, DC, P], bf16, tag="T1")
                nc.sync.dma_start_transpose(T1[:], lsb[:])
                nc.vector.tensor_tensor(
                    T1[:], T1[:],
                    w2T[:, :, None].to_broadcast((P, DC, P)), mult,
                )

                rsum = scal.tile([P, NS], f32, tag="rs")
                for h in range(NS):
                    ps = psS.tile([P, SW], f32, tag="psS")
                    for dc in range(DC):
                        for q in range(SW // MNS):
                            nc.tensor.matmul(
                                ps[:, q * MNS:(q + 1) * MNS],
                                T1[:, dc, :],
                                T2[:, dc, h * SW + q * MNS: h * SW + (q + 1) * MNS],
                                start=(dc == 0), stop=(dc == DC - 1),
                            )
                    nc.scalar.activation(
                        E[:, i, h * SW:(h + 1) * SW], ps[:], Exp,
                        accum_out=rsum[:, h:h + 1],
                    )

                ET = work.tile([P, RC, P], bf16, tag="ET")
                nc.sync.dma_start_transpose(ET[:], E[:, i, :])

                pc1 = psC1.tile([P, D], f32, tag="psC1")
                for k in range(RC):
                    for q in range(D // MN):
                        nc.tensor.matmul(
                            pc1[:, q * MN:(q + 1) * MN],
                            ET[:, k, :],
                            rhsb[:, k, q * MN:(q + 1) * MN],
                            start=(k == 0), stop=(k == RC - 1),
                        )

                rrec = scal.tile([P, 1], f32, tag="rrec")
                if NS > 1:
                    rtot = scal.tile([P, 1], f32, tag="rtot")
                    nc.vector.tensor_reduce(rtot[:], rsum[:], mybir.AxisListType.X, add)
                    nc.vector.reciprocal(rrec[:], rtot[:])
                else:
                    nc.vector.reciprocal(rrec[:], rsum[:])

                c1o = outp.tile([P, D], f32, tag="ctxo")
                nc.scalar.activation(c1o[:], pc1[:], Copy, scale=rrec[:])
                nc.sync.dma_start(out_lhs[i * P:(i + 1) * P, D:2 * D], c1o[:])

        # Phase C2: per r-chunk: rhs_ctx (E chunks read straight from SBUF)
        with tc.tile_pool(name="psC2", bufs=3, space="PSUM") as psC2, \
             tc.tile_pool(name="pscol", bufs=2, space="PSUM") as pscol:
            for k in range(RC):
                pc2 = psC2.tile([P, D], f32, tag="psC2")
                pcol = pscol.tile([P, 1], f32, tag="pcol")
                for i in range(LC):
                    ech = E[:, i, k * P:(k + 1) * P]
                    for q in range(D // MN):
                        nc.tensor.matmul(
                            pc2[:, q * MN:(q + 1) * MN],
                            ech,
                            lhsb[:, i, q * MN:(q + 1) * MN],
                            start=(i == 0), stop=(i == LC - 1),
                        )
                    nc.tensor.matmul(
                        pcol[:], ech, onesb[:],
                        start=(i == 0), stop=(i == LC - 1),
                    )
                crec = scal.tile([P, 1], f32, tag="crec")
                nc.vector.reciprocal(crec[:], pcol[:])
                c2o = outp.tile([P, D], f32, tag="ctxo")
                nc.scalar.activation(c2o[:], pc2[:], Copy, scale=crec[:])
                nc.sync.dma_start(out_rhs[k * P:(k + 1) * P, D:2 * D], c2o[:])

    nc.compile()
    return nc


_program = None


def _get_program():
    global _program
    if _program is None:
        _program = build_program(L, R, D)
    return _program


def kernel(lhs, rhs, w_lhs, w_rhs):
    from concourse.bass_utils import run_bass_kernel_spmd

    lhs = np.asarray(lhs, dtype=np.float32)
    rhs = np.asarray(rhs, dtype=np.float32)
    wl = np.asarray(w_lhs, dtype=np.float32).reshape(1, D)
    wr = np.asarray(w_rhs, dtype=np.float32).reshape(1, D)

    nc = _get_program()
    in_maps = [
        {"lhs": np.ascontiguousarray(lhs[c]), "rhs": np.ascontiguousarray(rhs[c]),
         "w_lhs": wl, "w_rhs": wr}
        for c in range(N_CORES)
    ]
    res = run_bass_kernel_spmd(nc, in_maps, core_ids=list(range(N_CORES)))
    out_lhs = np.stack([res.results[c]["out_lhs"] for c in range(N_CORES)])
    out_rhs = np.stack([res.results[c]["out_rhs"] for c in range(N_CORES)])
    return out_lhs, out_rhs
